# revision 1
# baseline (speedup 1.0000x reference)
"""Trainium2 Bass kernel for nn_AggregationGNN (edge-parallel GNN message passing).

Strategy (8 NeuronCores, SPMD):
  - Edges are owned by the core that owns their dst node (dst // 62500).
  - Per core, edges are bucketed by src chunk (8 chunks of 65536 rows, signed
    int16 gather indices with a +32768 base), and within each chunk sorted by
    dst and packed into fixed 1024-node dst windows (cap_w edges per window,
    3ish tiles of 128).
  - Per window: ucode dma_gather pulls atom_repr rows; bond embeddings are
    computed with a broadcast matmul (one-hot + RBF features synthesized from
    per-edge codes) + is_equal + exp; messages are scatter-added into a PSUM
    window via one-hot matmuls and retired into an SBUF-resident fp16
    aggregate; a fused MLP phase then produces the output tile by tile.
"""
import sys
sys.path.insert(0, "/opt/trn_rl_repo")

import numpy as np
import ml_dtypes

import concourse.bass as bass
import concourse.bacc as bacc
import concourse.mybir as mybir
import concourse.tile as tile
from concourse.library_config import mlp as mlp_lib

F32 = mybir.dt.float32
FP16 = mybir.dt.float16
BF16 = mybir.dt.bfloat16
I16 = mybir.dt.int16

N_NODES = 500000
N_EDGES = 1000000
D = 32
CD = 64
HID = 128
NCORE = 8
NPC = N_NODES // NCORE          # 62500 nodes per core
NCHUNK = 8                      # src chunks of 65536 (signed-idx trick)
CHUNK = 65536
WNODES = 1024                   # dst window width (nodes)
NWIN = (NPC + WNODES - 1) // WNODES   # 62 windows
AGGC = NWIN * WNODES            # padded agg columns (63488)
RBF_CENTERS = np.arange(0.0, 2.0, 0.1, dtype=np.float64)  # 20
NRBF = 20
RBF_GAMMA = 10.0
SENT = -20000.0                 # dst_rel sentinel for pad edges

# feature-row layout of the broadcast matmul output B [97, e]:
#   0:28   one-hot r (dir 0:8, type 8:24, ring 24:28)
#   28:56  one-hot p
#   56:64  unused (zero)
#   64:84  rbf exp-args r
#   84:104 rbf exp-args p
#   104    ones
NFEAT = 105

# moving payload rows [18, e] (bf16):
#  0 cd_r  1 ct_r  2 cr_r  3 cd_p  4 ct_p  5 cr_p
#  6 lh_r  7 lh_r  8 ll_r  9 l2h_r 10 l2l_r
# 11 lh_p 12 lh_p 13 ll_p 14 l2h_p 15 l2l_p
# 16 one  17 one
NPAY = 18


def _split_hi_lo(x):
    hi = x.astype(ml_dtypes.bfloat16).astype(np.float64)
    lo = (x - hi).astype(ml_dtypes.bfloat16).astype(np.float64)
    return hi, lo


def _build_const_S():
    """Stationary S [18, 97] bf16 for the broadcast matmul."""
    S = np.zeros((NPAY, NFEAT), np.float64)
    S[0, 0:8] = 1.0
    S[1, 8:24] = 1.0
    S[2, 24:28] = 1.0
    S[3, 28:36] = 1.0
    S[4, 36:52] = 1.0
    S[5, 52:56] = 1.0
    C = RBF_CENTERS
    tw = 2.0 * RBF_GAMMA * C            # 20*C
    twh, twl = _split_hi_lo(tw)
    q = -RBF_GAMMA * C * C              # -10*C^2
    qh, ql = _split_hi_lo(q)
    for base, (lh, lh2, ll, l2h, l2l) in ((64, (6, 7, 8, 9, 10)),
                                          (84, (11, 12, 13, 14, 15))):
        S[lh, base:base + NRBF] = twh
        S[lh2, base:base + NRBF] = twl
        S[ll, base:base + NRBF] = twh
        S[l2h, base:base + NRBF] = -RBF_GAMMA
        S[l2l, base:base + NRBF] = -RBF_GAMMA
        S[16, base:base + NRBF] = qh
        S[17, base:base + NRBF] = ql
    S[16, 104] = 1.0
    return S.astype(ml_dtypes.bfloat16)


def _build_const_M(emb_dir, emb_type, emb_ring, rbf_W, rbf_b):
    """M [97, 64] f32: bond matmul table producing concat_bond."""
    T = np.concatenate([emb_dir, emb_type, emb_ring], axis=0).astype(np.float64)  # [28, 32]
    W = rbf_W.astype(np.float64)        # [20, 32]
    M = np.zeros((NFEAT, CD), np.float64)
    M[0:28, 0:32] = T
    M[0:28, 32:64] = -T
    M[28:56, 32:64] = T
    M[64:84, 0:32] = W
    M[64:84, 32:64] = -W
    M[84:104, 32:64] = W
    M[104, 0:32] = rbf_b.astype(np.float64)
    return M.astype(np.float32)


def _prep_host(inputs):
    """Bucket/sort/pack edges; build all per-core device arrays."""
    src = np.asarray(inputs["src"]).astype(np.int64)
    dst = np.asarray(inputs["dst"]).astype(np.int64)
    core = dst // NPC
    dst_local = dst % NPC
    chunk = src >> 16
    win = dst_local // WNODES

    # order edges by (core, chunk, window, dst_local)
    order = np.lexsort((dst_local, win, chunk, core))
    o_core = core[order]
    o_chunk = chunk[order]
    o_win = win[order]

    # per (core, chunk, window) counts -> window cap
    key = (o_core * NCHUNK + o_chunk) * NWIN + o_win
    cnt = np.bincount(key, minlength=NCORE * NCHUNK * NWIN)
    cap_w = int(np.ceil((cnt.max() + 1) / 128.0) * 128)
    n_tile_w = cap_w // 128
    ecell = cap_w * NWIN                 # padded edges per (core, chunk)
    etot = ecell * NCHUNK                # padded edges per core
    ntile = etot // 128

    # slot for each edge: position within its (core,chunk,window) block
    starts = np.zeros(NCORE * NCHUNK * NWIN, np.int64)
    starts[1:] = np.cumsum(cnt)[:-1]
    rank = np.arange(len(order)) - starts[key]
    slot = key * cap_w + rank - (o_core * NCHUNK * NWIN) * cap_w  # within-core slot
    # (slot = ((chunk*NWIN + win) * cap_w) + rank)

    # per-core padded arrays
    gidx = np.zeros((NCORE, etot), np.int16)          # gather idx (signed)
    dst_rel = np.full((NCORE, etot), SENT, np.float32)
    pay = np.zeros((NCORE, NPAY, etot), ml_dtypes.bfloat16)

    oc = o_core.astype(np.int64)
    osl = slot.astype(np.int64)
    e = order
    gidx[oc, osl] = (src[e] - (chunk[e] << 16) - 32768).astype(np.int16)
    dst_rel[oc, osl] = (dst_local[e] - o_win * WNODES).astype(np.float32)

    r_dir = np.asarray(inputs["r_dir"])[e]
    r_type = np.asarray(inputs["r_type"])[e]
    r_ring = np.asarray(inputs["r_ring"])[e]
    p_dir = np.asarray(inputs["p_dir"])[e]
    p_type = np.asarray(inputs["p_type"])[e]
    p_ring = np.asarray(inputs["p_ring"])[e]
    r_len = np.asarray(inputs["r_len"]).astype(np.float64)[e]
    p_len = np.asarray(inputs["p_len"]).astype(np.float64)[e]

    def setpay(row, vals):
        pay[oc, row, osl] = vals.astype(ml_dtypes.bfloat16)

    setpay(0, r_dir.astype(np.float64))
    setpay(1, 8.0 + r_type)
    setpay(2, 24.0 + r_ring)
    setpay(3, 28.0 + p_dir)
    setpay(4, 36.0 + p_type)
    setpay(5, 52.0 + p_ring)
    for base_row, ln in ((6, r_len), (11, p_len)):
        lh, ll = _split_hi_lo(ln)
        l2h, l2l = _split_hi_lo(ln * ln)
        setpay(base_row, lh)
        setpay(base_row + 1, lh)
        setpay(base_row + 2, ll)
        setpay(base_row + 3, l2h)
        setpay(base_row + 4, l2l)
    pay[:, 16, :] = np.float64(1.0)
    pay[:, 17, :] = np.float64(1.0)
    # pad slots: codes 0 -> one-hot garbage? code 0 matches iota row 0 -> adds
    # emb row to bond, but dst_rel sentinel kills the one-hot so nothing lands.
    # pads gather idx 0 -> reads row 32768+0*65536 etc. (valid, finite).

    # within each 128-tile, put negative gather idxs first (the ucode trims
    # trailing negatives from each call; pads at the end have idx 0 >= 0)
    ntile_all = etot // 128
    for c in range(NCORE):
        gt = gidx[c].reshape(ntile_all, 128)
        dt_ = dst_rel[c].reshape(ntile_all, 128)
        pt = pay[c].reshape(NPAY, ntile_all, 128)
        ordt = np.argsort(gt >= 0, axis=1, kind="stable")
        gidx[c] = np.take_along_axis(gt, ordt, axis=1).reshape(-1)
        dst_rel[c] = np.take_along_axis(dt_, ordt, axis=1).reshape(-1)
        pay[c] = np.take_along_axis(pt, ordt[None], axis=2).reshape(NPAY, -1)

    # idx wrap layout per gather call (one call per (chunk, window) = cap_w idxs):
    # position i of call -> [i%16 (replicated x8), i//16]
    g = gidx.reshape(NCORE, NCHUNK * NWIN, cap_w)  # [NC, calls, cap_w]
    g = g.reshape(NCORE, NCHUNK * NWIN, cap_w // 16, 16)
    g = np.swapaxes(g, 2, 3)                                # [NC, calls, 16, cap/16]
    gwrap = np.tile(g, (1, 1, 8, 1))                        # [NC, calls, 128, cap/16]

    # dst_rel tiles: [NC, ntile, 128] -> device loads [128, n_tile_w] per window
    drel = dst_rel.reshape(NCORE, ntile, 128)
    drel = np.swapaxes(drel, 1, 2).copy()                   # [NC, 128, ntile]

    return dict(cap_w=cap_w, n_tile_w=n_tile_w, ecell=ecell, etot=etot,
                ntile=ntile, gwrap=gwrap, pay=pay, drel=drel)


_CACHE = {}


def _build_program(cap_w, n_tile_w):
    assert cap_w <= 512, cap_w
    nc = bacc.Bacc("TRN2", debug=False, num_swdge_queues=2,
                   dynamic_dma_scratch_size=1 << 15)
    NTW = n_tile_w
    NCALL = NCHUNK * NWIN

    atom = nc.dram_tensor("atom", [N_NODES, CD], F32, kind="ExternalInput")
    gidx_d = nc.dram_tensor("gidx", [NCALL, 128, cap_w // 16], I16,
                            kind="ExternalInput")
    pay_d = nc.dram_tensor("pay", [NPAY, NCALL * cap_w], BF16,
                           kind="ExternalInput")
    drel_d = nc.dram_tensor("drel", [128, NCALL * NTW], F32,
                            kind="ExternalInput")
    S_d = nc.dram_tensor("S", [NPAY, NFEAT], BF16, kind="ExternalInput")
    M_d = nc.dram_tensor("M", [NFEAT, CD], F32, kind="ExternalInput")
    iota97_d = nc.dram_tensor("iota97", [NFEAT, 1], F32, kind="ExternalInput")
    iotaW_d = nc.dram_tensor("iotaW", [128, WNODES], FP16, kind="ExternalInput")
    W1_d = nc.dram_tensor("W1", [CD, HID], FP16, kind="ExternalInput")
    W2_d = nc.dram_tensor("W2", [HID, CD], FP16, kind="ExternalInput")
    b1_d = nc.dram_tensor("b1", [HID, 1], F32, kind="ExternalInput")
    b2_d = nc.dram_tensor("b2", [1, CD], FP16, kind="ExternalInput")
    ones1_d = nc.dram_tensor("ones1", [1, 128], FP16, kind="ExternalInput")
    out_d = nc.dram_tensor("out", [NPC, CD], F32, kind="ExternalOutput")

    with tile.TileContext(nc) as tc:
        with tc.tile_pool(name="const", bufs=1) as cpool, \
             tc.tile_pool(name="agg", bufs=1) as apool, \
             tc.tile_pool(name="work", bufs=3) as wpool, \
             tc.tile_pool(name="stage", bufs=4) as spool, \
             tc.tile_pool(name="stage2", bufs=3) as wpool2:

            nc.gpsimd.load_library(mlp_lib)

            S_sb = cpool.tile([NPAY, NFEAT], BF16, name="S_sb")
            nc.sync.dma_start(out=S_sb[:], in_=S_d[:])
            M_sb = cpool.tile([NFEAT, CD], F32, name="M_sb")
            nc.sync.dma_start(out=M_sb[:], in_=M_d[:])
            iota97 = cpool.tile([NFEAT, 1], F32, name="iota97")
            nc.sync.dma_start(out=iota97[:], in_=iota97_d[:])
            iotaW = cpool.tile([128, WNODES], FP16, name="iotaW")
            nc.sync.dma_start(out=iotaW[:], in_=iotaW_d[:])
            W1_sb = cpool.tile([CD, HID], FP16, name="W1_sb")
            nc.sync.dma_start(out=W1_sb[:], in_=W1_d[:])
            W2_sb = cpool.tile([HID, CD], FP16, name="W2_sb")
            nc.sync.dma_start(out=W2_sb[:], in_=W2_d[:])
            b1_sb = cpool.tile([HID, 1], F32, name="b1_sb")
            nc.sync.dma_start(out=b1_sb[:], in_=b1_d[:])
            b2_sb = cpool.tile([1, CD], FP16, name="b2_sb")
            nc.sync.dma_start(out=b2_sb[:], in_=b2_d[:])
            ones1 = cpool.tile([1, 128], FP16, name="ones1")
            nc.sync.dma_start(out=ones1[:], in_=ones1_d[:])

            agg = apool.tile([CD, AGGC], FP16, name="aggbuf")
            nc.vector.memset(agg[:], 0.0)

            NSTG = 4
            stg = [spool.tile([128, NTW, CD], F32, tag=f"stg{i}", name=f"stg{i}")
                   for i in range(NSTG)]
            gsem = [nc.alloc_semaphore(f"gsem{i}") for i in range(NSTG)]
            used = [0] * NSTG

            # ---------------- edge phase ----------------
            psp_cm = tc.tile_pool(name="ps", bufs=2, space="PSUM")
            pswp_cm = tc.tile_pool(name="psw", bufs=2, space="PSUM")
            psp = psp_cm.__enter__()
            pswp = pswp_cm.__enter__()
            if True:
                for c in range(NCHUNK):
                    abase = 32768 + c * CHUNK
                    row0 = min(abase, N_NODES - 1)
                    for w in range(NWIN):
                        call = c * NWIN + w
                        b = call % NSTG
                        # load idx tile and issue gather
                        gi = wpool.tile([128, cap_w // 16], I16, tag="gi", name="gi")
                        nc.sync.dma_start(out=gi[:], in_=gidx_d[call])
                        with tc.tile_critical():
                            if used[b] > 0:
                                nc.gpsimd.wait_ge(gsem[b], 16 * used[b])
                            nc.gpsimd.dma_gather(
                                out_ap=stg[b][:], in_ap=atom[row0:row0 + 1, :],
                                idxs_ap=gi[:], num_idxs=cap_w, num_idxs_reg=cap_w,
                                elem_size=CD, queue_num=call % 2,
                            ).then_inc(gsem[b], 16)
                        used[b] += 1

                        # payload + dst_rel
                        payt = wpool.tile([NPAY, cap_w], BF16, tag="payt", name="payt")
                        nc.sync.dma_start(
                            out=payt[:], in_=pay_d[:, call * cap_w:(call + 1) * cap_w])
                        drt = wpool.tile([128, NTW], F32, tag="drt", name="drt")
                        nc.sync.dma_start(
                            out=drt[:], in_=drel_d[:, call * NTW:(call + 1) * NTW])

                        # broadcast matmul -> B psum [97, cap_w]
                        Bp = psp.tile([NFEAT, cap_w], F32, tag="Bp", name="Bp",
                                      space="PSUM")
                        nc.tensor.matmul(Bp[:], S_sb[:], payt[:], start=True, stop=True)
                        # F features
                        Ft = wpool.tile([NFEAT, cap_w], F32, tag="Ft", name="Ft")
                        nc.vector.tensor_scalar(
                            out=Ft[:], in0=Bp[:], scalar1=iota97[:], scalar2=None,
                            op0=mybir.AluOpType.is_equal)
                        nc.scalar.activation(Ft[64:104, :], Bp[64:104, :],
                                             mybir.ActivationFunctionType.Exp)
                        # bond matmuls -> bond psum [128, NTW*64]
                        bp = psp.tile([128, NTW * CD], F32, tag="bp", name="bp",
                                      space="PSUM")
                        for ti in range(NTW):
                            nc.tensor.matmul(
                                bp[:, ti * CD:(ti + 1) * CD],
                                Ft[:, ti * 128:(ti + 1) * 128],
                                M_sb[:], start=True, stop=True)
                        # msg = atom + bond (DVE; also the gather sync point)
                        msg = wpool.tile([128, NTW, CD], FP16, tag="msg", name="msg")
                        with tc.tile_critical():
                            nc.vector.wait_ge(gsem[b], 16 * used[b])
                            nc.vector.tensor_tensor(
                                out=msg[:].rearrange("p t d -> p (t d)"),
                                in0=stg[b][:].rearrange("p t d -> p (t d)"),
                                in1=bp[:],
                                op=mybir.AluOpType.add)
                        # scatter matmuls into window psum
                        wp = pswp.tile([CD, WNODES], F32, tag="wp", name="wp",
                                       space="PSUM")
                        for ti in range(NTW):
                            oh = wpool.tile([128, WNODES], FP16, tag="oh", name="oh")
                            nc.vector.tensor_scalar(
                                out=oh[:], in0=iotaW[:], scalar1=drt[:, ti:ti + 1],
                                scalar2=None, op0=mybir.AluOpType.is_equal)
                            for half in range(WNODES // 512):
                                nc.tensor.matmul(
                                    wp[:, half * 512:(half + 1) * 512],
                                    msg[:, ti, :],
                                    oh[:, half * 512:(half + 1) * 512],
                                    start=(ti == 0), stop=(ti == NTW - 1))
                        # retire window into agg
                        nc.vector.tensor_tensor(
                            out=agg[:, w * WNODES:(w + 1) * WNODES],
                            in0=wp[:],
                            in1=agg[:, w * WNODES:(w + 1) * WNODES],
                            op=mybir.AluOpType.add)

            pswp_cm.__exit__(None, None, None)
            psp_cm.__exit__(None, None, None)
            # ---------------- MLP phase ----------------
            mlp_ps_cm = tc.tile_pool(name="mlps", bufs=2, space="PSUM")
            psp = mlp_ps_cm.__enter__()
            NB = 4  # node tiles per batch (512 nodes)
            for s in range(0, NPC, 128 * NB):
                nb = min(NB, (NPC - s + 127) // 128)
                cols = nb * 128
                hp = psp.tile([HID, NB * 128], F32, tag="hp", name="hp",
                              space="PSUM")
                nc.tensor.matmul(hp[:, :cols], W1_sb[:], agg[:, s:s + cols],
                                 start=True, stop=True)
                hT = wpool.tile([HID, NB * 128], FP16, tag="hT", name="hT")
                nc.scalar.activation(hT[:, :cols], hp[:, :cols],
                                     mybir.ActivationFunctionType.Relu,
                                     bias=b1_sb[:])
                op = psp.tile([128, NB * CD], F32, tag="op", name="op",
                              space="PSUM")
                for t in range(nb):
                    nc.tensor.matmul(op[:, t * CD:(t + 1) * CD],
                                     hT[:, t * 128:(t + 1) * 128], W2_sb[:],
                                     start=True, stop=False)
                    nc.tensor.matmul(op[:, t * CD:(t + 1) * CD],
                                     ones1[:], b2_sb[:],
                                     start=False, stop=True)
                ot = wpool.tile([128, NB * CD], F32, tag="ot", name="ot")
                nc.scalar.activation(ot[:, :nb * CD], op[:, :nb * CD],
                                     mybir.ActivationFunctionType.Relu)
                for t in range(nb):
                    if s + t * 128 >= NPC:
                        break
                    rows = min(128, NPC - (s + t * 128))
                    nc.sync.dma_start(
                        out=out_d[s + t * 128: s + t * 128 + rows, :],
                        in_=ot[:rows, t * CD:(t + 1) * CD])
            mlp_ps_cm.__exit__(None, None, None)

    nc.compile()
    return nc


def _make_exec(nc):
    import jax
    from concourse import bass2jax
    from concourse.bass2jax import _bass_exec_p, install_neuronx_cc_hook
    from jax.sharding import Mesh, PartitionSpec
    from jax.experimental.shard_map import shard_map
    import concourse.mybir as mb
    install_neuronx_cc_hook()

    in_names, out_names, out_avals, zero_outs = [], [], [], []
    pname = nc.partition_id_tensor.name if nc.partition_id_tensor else None
    for alloc in nc.m.functions[0].allocations:
        if not isinstance(alloc, mb.MemoryLocationSet):
            continue
        name = alloc.memorylocations[0].name
        if alloc.kind == "ExternalInput":
            if name != pname:
                in_names.append(name)
        elif alloc.kind == "ExternalOutput":
            out_names.append(name)
            shape = tuple(alloc.tensor_shape)
            dtype = mb.dt.np(alloc.dtype)
            out_avals.append(jax.core.ShapedArray(shape, dtype))
            zero_outs.append(np.zeros(shape, dtype))
    n_params = len(in_names)
    all_in = in_names + out_names + ([pname] if pname else [])

    def _body(*args):
        ops = list(args)
        if pname is not None:
            ops.append(bass2jax.partition_id_tensor())
        return tuple(_bass_exec_p.bind(
            *ops, out_avals=tuple(out_avals), in_names=tuple(all_in),
            out_names=tuple(out_names), lowering_input_output_aliases=(),
            sim_require_finite=True, sim_require_nnan=True, nc=nc))

    donate = tuple(range(n_params, n_params + len(out_names)))
    devices = jax.devices()[:NCORE]
    mesh = Mesh(np.asarray(devices), ("core",))
    in_specs = (PartitionSpec("core"),) * (n_params + len(out_names))
    out_specs = (PartitionSpec("core"),) * len(out_names)
    sharded = jax.jit(
        shard_map(_body, mesh=mesh, in_specs=in_specs, out_specs=out_specs,
                  check_rep=False),
        donate_argnums=donate, keep_unused=True)
    return dict(fn=sharded, in_names=in_names, out_names=out_names,
                out_avals=out_avals, zero_outs=zero_outs)


def _run(nc, in_maps, ex=None, time_iters=0):
    import jax, time as _time
    if ex is None:
        ex = _make_exec(nc)
    in_names, out_names = ex["in_names"], ex["out_names"]
    out_avals, zero_outs = ex["out_avals"], ex["zero_outs"]
    per_core = [[np.asarray(m[n]) for n in in_names] for m in in_maps]
    concat_in = [np.concatenate([per_core[c][i] for c in range(NCORE)], axis=0)
                 for i in range(len(in_names))]
    concat_zeros = [np.zeros((NCORE * z.shape[0], *z.shape[1:]), z.dtype)
                    for z in zero_outs]
    out_arrs = ex["fn"](*concat_in, *concat_zeros)
    result = [
        {name: np.asarray(out_arrs[i]).reshape(NCORE, *out_avals[i].shape)[c]
         for i, name in enumerate(out_names)}
        for c in range(NCORE)
    ]
    times = None
    if time_iters:
        dev_in = [jax.device_put(a) for a in concat_in]
        zsets = [[jax.device_put(np.zeros((NCORE * z.shape[0], *z.shape[1:]),
                                          z.dtype)) for z in zero_outs]
                 for _ in range(time_iters)]
        times = []
        for it in range(time_iters):
            t0 = _time.time()
            o = ex["fn"](*dev_in, *zsets[it])
            jax.block_until_ready(o)
            times.append(_time.time() - t0)
    return result, times


_TRIV = {}


def _trivial_overhead_ns(iters=8):
    """Calibrate the fixed axon dispatch overhead with a near-empty kernel."""
    import jax, time as _time
    if "ex" not in _TRIV:
        nc = bacc.Bacc("TRN2", debug=False)
        a = nc.dram_tensor("a", [128, 128], F32, kind="ExternalInput")
        o = nc.dram_tensor("o", [128, 128], F32, kind="ExternalOutput")
        with tile.TileContext(nc) as tc:
            with tc.tile_pool(name="p", bufs=1) as pool:
                t = pool.tile([128, 128], F32)
                nc.sync.dma_start(out=t[:], in_=a[:])
                nc.sync.dma_start(out=o[:], in_=t[:])
        nc.compile()
        _TRIV["nc"] = nc
        _TRIV["ex"] = _make_exec(nc)
    ex = _TRIV["ex"]
    a_np = np.zeros((NCORE * 128, 128), np.float32)
    dev_in = [jax.device_put(a_np)]
    zsets = [[jax.device_put(np.zeros((NCORE * 128, 128), np.float32))]
             for _ in range(iters + 1)]
    out = ex["fn"](dev_in[0], zsets[0][0])
    jax.block_until_ready(out)
    ts = []
    for i in range(iters):
        t0 = _time.time()
        out = ex["fn"](dev_in[0], zsets[i + 1][0])
        jax.block_until_ready(out)
        ts.append(_time.time() - t0)
    ts.sort()
    return ts[len(ts) // 2] * 1e9, ts


def kernel(**inputs):
    prep = _prep_host(inputs)
    cap_w, n_tile_w = prep["cap_w"], prep["n_tile_w"]

    key = (cap_w, n_tile_w)
    if key not in _CACHE:
        _CACHE[key] = _build_program(cap_w, n_tile_w)
    nc = _CACHE[key]

    S = np.asarray(_build_const_S())
    M = _build_const_M(np.asarray(inputs["emb_dir"]), np.asarray(inputs["emb_type"]),
                       np.asarray(inputs["emb_ring"]), np.asarray(inputs["rbf_W"]),
                       np.asarray(inputs["rbf_b"]))
    iota97 = np.full((NFEAT, 1), -1e9, np.float32)
    iota97[0:56, 0] = np.arange(56)
    iota97[104, 0] = 1.0
    iotaW = np.tile(np.arange(WNODES, dtype=np.float16), (128, 1))
    atom = np.asarray(inputs["atom_repr"]).astype(np.float32)
    W1 = np.asarray(inputs["W1"]).astype(np.float16)
    W2 = np.asarray(inputs["W2"]).astype(np.float16)
    b1 = np.asarray(inputs["b1"]).astype(np.float32).reshape(HID, 1)
    b2 = np.asarray(inputs["b2"]).astype(np.float16).reshape(1, CD)
    ones1 = np.ones((1, 128), np.float16)

    in_maps = []
    for c in range(NCORE):
        in_maps.append({
            "atom": atom,
            "gidx": prep["gwrap"][c],
            "pay": np.asarray(prep["pay"][c]),
            "drel": prep["drel"][c].astype(np.float32),
            "S": S, "M": M, "iota97": iota97, "iotaW": iotaW,
            "W1": W1, "W2": W2, "b1": b1, "b2": b2, "ones1": ones1,
        })

    if key not in _CACHE or not isinstance(_CACHE[key], dict):
        _CACHE[key] = {"nc": nc, "ex": _make_exec(nc)}
    results, _ = _run(nc, in_maps, ex=_CACHE[key]["ex"])
    out = np.concatenate([results[c]["out"] for c in range(NCORE)], axis=0)
    return out.astype(np.float32)


def _assemble_in_maps(inputs, prep):
    S = np.asarray(_build_const_S())
    M = _build_const_M(np.asarray(inputs["emb_dir"]), np.asarray(inputs["emb_type"]),
                       np.asarray(inputs["emb_ring"]), np.asarray(inputs["rbf_W"]),
                       np.asarray(inputs["rbf_b"]))
    iota97 = np.full((NFEAT, 1), -1e9, np.float32)
    iota97[0:56, 0] = np.arange(56)
    iota97[104, 0] = 1.0
    iotaW = np.tile(np.arange(WNODES, dtype=np.float16), (128, 1))
    atom = np.asarray(inputs["atom_repr"]).astype(np.float32)
    W1 = np.asarray(inputs["W1"]).astype(np.float16)
    W2 = np.asarray(inputs["W2"]).astype(np.float16)
    b1 = np.asarray(inputs["b1"]).astype(np.float32).reshape(HID, 1)
    b2 = np.asarray(inputs["b2"]).astype(np.float16).reshape(1, CD)
    ones1 = np.ones((1, 128), np.float16)
    in_maps = []
    for c in range(NCORE):
        in_maps.append({
            "atom": atom,
            "gidx": prep["gwrap"][c],
            "pay": np.asarray(prep["pay"][c]),
            "drel": prep["drel"][c].astype(np.float32),
            "S": S, "M": M, "iota97": iota97, "iotaW": iotaW,
            "W1": W1, "W2": W2, "b1": b1, "b2": b2, "ones1": ones1,
        })
    return in_maps


def time_kernel(iters=8, **inputs):
    """Run the compiled kernel repeatedly with device-resident inputs and
    return (median_exec_ns_minus_overhead, raw_times, overhead_times)."""
    prep = _prep_host(inputs)
    cap_w, n_tile_w = prep["cap_w"], prep["n_tile_w"]
    key = (cap_w, n_tile_w)
    if key not in _CACHE or not isinstance(_CACHE[key], dict):
        nc = _build_program(cap_w, n_tile_w)
        _CACHE[key] = {"nc": nc, "ex": _make_exec(nc)}
    in_maps = _assemble_in_maps(inputs, prep)
    _, times = _run(_CACHE[key]["nc"], in_maps, ex=_CACHE[key]["ex"],
                    time_iters=iters)
    over_ns, over_ts = _trivial_overhead_ns()
    times.sort()
    med = times[len(times) // 2] * 1e9
    return max(0.0, med - over_ns), times, over_ts

